# revision 28
# baseline (speedup 1.0000x reference)
"""Trainium2 Bass kernel for nn_Attention_59708635349115 (v3).

Decoder self-attention (GQA 16 q-heads / 4 kv-heads, RoPE, causal) over
B=2, S=2048, H=2048, distributed over 8 NeuronCores as 2 (batch) x 4
(head-group) shards.  Each core computes q/k/v projections for its 4
q-heads / 1 kv-head, causal attention, and a partial o-projection against
its 512-row slice of Wo; the host sums the 4 partials per batch.

v3 changes over v2 (one fused software-pipelined stream, bf16 data path):
  - cross-body pipelining: the timing loop unrolls the body 4x inside the
    hardware loop, and the instruction stream now wraps around body
    boundaries — the next body's x loads, k/v/q0 projections, and weight
    reloads ride as PE filler inside the previous body's last q-chunk, so
    the per-body DMA preamble stall (and the HAM re-throttle it caused)
    only happens once per hardware-loop iteration.  kT/v/wq/wk/wv/cos/sin
    are double-buffered to break the cross-body WAR serialization.
  - o-projection as filler: emit_oproj is a generator pumped into the
    next q-chunk's attention stream instead of a dense block, so the PE
    never idles at q-chunk boundaries and the softmax-denominator tiny
    matmuls always have a long matmul in front to hide their LDWEIGHTS.
"""

import os
import sys
from collections import deque

for _p in ("/opt/trn_rl_repo", "/root/.axon_site/_ro/trn_rl_repo"):
    if os.path.isdir(_p) and _p not in sys.path:
        sys.path.insert(0, _p)

import numpy as np
import ml_dtypes

import concourse.bass as bass
import concourse.mybir as mybir
import concourse.tile as tile
from concourse import bacc
from concourse.bass_utils import run_bass_kernel_spmd

B, S, H = 2, 2048, 2048
NH, NKV = 16, 4
HD = H // NH            # 128
G = 4                   # head-group shards (tensor parallel)
HPC = NH // G           # 4 q heads per core
N_CORES = 8
P = 128                 # partition dim
NQ = 512                # q-chunk (matmul moving dim)
NJ = S // NQ            # 4 q-chunks
KC = S // P             # 16 key/token 128-chunks
HC = H // P             # 16 hidden 128-chunks
NS = NQ // P            # 4 128-subchunks per q-chunk

F32 = mybir.dt.float32
BF16 = mybir.dt.bfloat16
AF = mybir.ActivationFunctionType
BF = ml_dtypes.bfloat16

_CACHE = {}


def _build_program(loop_n=1, unroll=None):
    if unroll is None:
        unroll = 1
        if loop_n > 1:
            for u in (16, 8, 4, 2):
                if loop_n % u == 0:
                    unroll = u
                    break
    nc = bacc.Bacc("TRN2", target_bir_lowering=False, debug=False,
                   num_devices=N_CORES)

    ext = {}
    for name, shape, dt in [
        ("xT", [H, S], BF16),
        ("wq", [P, HC * HPC * HD], BF16),   # host pre-arranged SBUF layout
        ("wk", [P, HC * HD], BF16),
        ("wv", [P, HC * HD], BF16),
        ("wo", [HPC * HD, H], BF16),
        ("cosT", [HD, S], BF16),
        ("sinnegT", [HD, S], BF16),
        ("tri", [P, P], BF16),
        ("identb", [P, P], BF16),
        ("mbias", [P, KC], F32),
        ("onescol", [P, 1], BF16),
    ]:
        ext[name] = nc.dram_tensor(name, shape, dt, kind="ExternalInput")
    out_ext = nc.dram_tensor("out_p", [S, H], BF16, kind="ExternalOutput")

    scale = float(1.0 / np.sqrt(HD))
    NSTEP = unroll * NJ

    from contextlib import nullcontext
    with nc.allow_low_precision(reason="bf16 data path is intended"), \
         tile.TileContext(nc) as tc:
        with tc.tile_pool(name="persist", bufs=1) as persist, \
             (tc.For_i(0, loop_n // unroll, 1,
                       hint_engines=(mybir.EngineType.PE,
                                     mybir.EngineType.Activation,
                                     mybir.EngineType.DVE,
                                     mybir.EngineType.Pool,
                                     mybir.EngineType.SP))
              if loop_n > 1 else nullcontext()):
            # consts: loaded once per hardware-loop iteration (tiny)
            tri_sb = persist.tile([P, P], BF16)
            identb_sb = persist.tile([P, P], BF16)
            ones_sb = persist.tile([P, 1], BF16)
            mb_sb = persist.tile([P, KC], F32)
            wo_sb = persist.tile([P, HPC * H], BF16)     # [hd, h*H + m]

            from contextlib import ExitStack
            with ExitStack() as _stack:
                _p = lambda *a, **k: _stack.enter_context(
                    tc.tile_pool(*a, **k))
                kvp = _p(name="kv", bufs=2)
                wqp = _p(name="wqp", bufs=2)
                wkp = _p(name="wkp", bufs=2)
                wvp = _p(name="wvp", bufs=2)
                trigp = _p(name="trigp", bufs=2)
                xtp = _p(name="xt", bufs=2)
                rope = _p(name="rope", bufs=3)
                qtp = _p(name="qt", bufs=8)
                vtp = _p(name="vt", bufs=2)
                etp = _p(name="et", bufs=6)
                normp = _p(name="norm", bufs=2)
                outTp = _p(name="outT", bufs=8)
                stp = _p(name="st", bufs=3)
                psB = _p(name="psB", bufs=5, space="PSUM")
                psO = _p(name="psO", bufs=2, space="PSUM")

                nc.scalar.dma_start(tri_sb[:], ext["tri"][:])
                nc.scalar.dma_start(ones_sb[:], ext["onescol"][:])
                nc.scalar.dma_start(mb_sb[:], ext["mbias"][:])
                nc.scalar.dma_start(identb_sb[:], ext["identb"][:])

                # ---- per-body double-buffered SBUF state ----
                class Body:
                    pass

                bodies = [None, None]    # by u: Body with tiles

                def alloc_body(u):
                    b = Body()
                    b.kT = kvp.tile([P, S], BF16, tag="kT",
                                    name=f"kT_{u}")
                    b.v = kvp.tile([P, S], BF16, tag="v", name=f"v_{u}")
                    b.wq = wqp.tile([P, HC * HPC * HD], BF16, tag="wq",
                                    name=f"wq_{u}")
                    b.wk = wkp.tile([P, HC * HD], BF16, tag="wk",
                                    name=f"wk_{u}")
                    b.wv = wvp.tile([P, HC * HD], BF16, tag="wv",
                                    name=f"wv_{u}")
                    b.cos = trigp.tile([HD, S], BF16, tag="cos",
                                       name=f"cos_{u}")
                    b.sin = trigp.tile([HD, S], BF16, tag="sin",
                                       name=f"sin_{u}")
                    return b

                # -------- DMA helpers (Activation DGE queue = bulk) -------
                def load_wk_wv(b, eng=None):
                    eng = eng or nc.scalar
                    half = HC * HD // 2
                    for part in range(2):
                        eng.dma_start(
                            b.wk[:, part * half:(part + 1) * half],
                            ext["wk"][:, part * half:(part + 1) * half])
                    for part in range(2):
                        eng.dma_start(
                            b.wv[:, part * half:(part + 1) * half],
                            ext["wv"][:, part * half:(part + 1) * half])

                def load_trig(b, j0, j1, eng=None):
                    eng = eng or nc.scalar
                    eng.dma_start(
                        b.cos[:, j0 * NQ:j1 * NQ],
                        ext["cosT"][:, j0 * NQ:j1 * NQ])
                    eng.dma_start(
                        b.sin[:, j0 * NQ:j1 * NQ],
                        ext["sinnegT"][:, j0 * NQ:j1 * NQ])

                def load_wq_head(b, h, eng=None):
                    eng = eng or nc.scalar
                    w = HC * HD
                    eng.dma_start(
                        b.wq[:, h * w:(h + 1) * w],
                        ext["wq"][:, h * w:(h + 1) * w])

                def load_wo_part(h, eng=None):
                    eng = eng or nc.scalar
                    eng.dma_start(
                        wo_sb[:, h * H:(h + 1) * H],
                        ext["wo"][h * P:(h + 1) * P, :])

                # x chunk loads: SP queue by default (Act for the short
                # jq==0 steps, whose SP queue must stay clear for the rope
                # rot copies), 4 parts per step
                xt_parts_emitted = {}

                def load_xt_part(xt_tile, jq, p4, eng=None):
                    eng = eng or nc.sync
                    c0 = p4 * (HC // 4)
                    eng.dma_start(
                        xt_tile[:, c0 * NQ:(c0 + HC // 4) * NQ],
                        ext["xT"][c0 * P:(c0 + HC // 4) * P,
                                  jq * NQ:(jq + 1) * NQ]
                        .rearrange("(c p) t -> p c t", p=P))
                    key = xt_tile.tensor.name
                    xt_parts_emitted[key] = max(
                        xt_parts_emitted.get(key, 0), p4 + 1)

                # ---------------- PE filler machinery ----------------
                # hi: q-head projections for the current step (latency
                #     critical: their rope chain gates the next attn head).
                # mid: next-step k/v/q0 projections (gate the next step's
                #     attention; flushed at step end).
                # oq: previous step's o-projection (ready bulk filler).
                fill_hi = deque()
                fill_mid = deque()
                fill_oq = deque()

                def _pump_q(q):
                    while q:
                        try:
                            r = next(q[0])
                            return "stall" if r == "stall" else "ok"
                        except StopIteration:
                            q.popleft()
                    return "empty"

                def pump(n=1):
                    while n > 0:
                        for q in (fill_hi, fill_mid, fill_oq):
                            if _pump_q(q) == "ok":
                                break
                        else:
                            return
                        n -= 1

                def flush(gen):
                    for q in (fill_hi, fill_mid, fill_oq):
                        while any(g is gen for g in q):
                            if _pump_q(q) == "stall":
                                raise RuntimeError(
                                    "flush of a gated generator stalled")

                def flush_all():
                    while True:
                        r = _pump_q(fill_hi)
                        if r == "empty":
                            r = _pump_q(fill_mid)
                        if r == "empty":
                            r = _pump_q(fill_oq)
                        if r == "empty":
                            return
                        if r == "stall":
                            raise RuntimeError("flush_all stalled")

                def gen_proj(w_sb, col0, stride, xt_tile, sink, gated=False):
                    ps = psB.tile([P, NQ], F32, tag="big", name="ps_proj")
                    key = xt_tile.tensor.name if gated else None
                    for c in range(HC):
                        while gated and \
                                xt_parts_emitted.get(key, 0) * (HC // 4) <= c:
                            yield "stall"
                        base = c * stride + col0
                        nc.tensor.matmul(
                            ps[:], w_sb[:, base:base + HD],
                            xt_tile[:, c * NQ:(c + 1) * NQ],
                            start=(c == 0), stop=(c == HC - 1))
                        yield "ok"
                    sink(ps)

                def rope_sink(b, dst_ap, jq, on_act=False):
                    def sink(ps):
                        raw = rope.tile([P, NQ], BF16, tag="raw")
                        if on_act:   # Act is exp-light at small jq
                            nc.scalar.activation(raw[:], ps[:], AF.Copy)
                        else:
                            nc.vector.tensor_copy(raw[:], ps[:])
                        rot = rope.tile([P, NQ], BF16, tag="rot")
                        half = HD // 2
                        nc.sync.dma_start(rot[0:half, :], raw[half:P, :])
                        nc.sync.dma_start(rot[half:P, :], raw[0:half, :])
                        t1 = rope.tile([P, NQ], BF16, tag="t1")
                        nc.vector.tensor_mul(
                            t1[:], raw[:], b.cos[:, jq * NQ:(jq + 1) * NQ])
                        t2 = rope.tile([P, NQ], BF16, tag="t2")
                        nc.vector.tensor_mul(
                            t2[:], rot[:], b.sin[:, jq * NQ:(jq + 1) * NQ])
                        nc.vector.tensor_add(dst_ap, t1[:], t2[:])
                    return sink

                def v_sink(b, jq):
                    def sink(ps):
                        vt_sb = vtp.tile([P, NQ], BF16, tag="vt")
                        nc.vector.tensor_copy(vt_sb[:], ps[:])
                        for s4 in range(NS):
                            kc = jq * NS + s4
                            nc.sync.dma_start(
                                b.v[:, kc * P:(kc + 1) * P],
                                vt_sb[:, s4 * P:(s4 + 1) * P],
                                transpose=True)
                    return sink

                def attn_head(b, h, jq, qt, outT_h, finish_prev):
                    """finish_prev = (finish_a, finish_b) of the previous
                    head: a (recip/transpose/broadcast prep) runs at kc==1,
                    b (the final DVE outT multiply, which waits on the Pool
                    broadcasts) is deferred to the end of this head so it
                    cannot head-of-line-block this head's DVE tri-masks."""
                    nkc = (jq + 1) * NS
                    ps_out = psO.tile([P, NQ], F32, tag="o2")
                    ps_d = psO.tile([P, NS], F32, tag="d", bufs=1)
                    pend = deque()
                    # off-diagonal et chunks are summed in bf16 pairs on
                    # the DVE, halving the denominator tiny-matmul count;
                    # the diagonal chunks keep per-chunk tiny matmuls
                    n_tiny = (jq * NS // 2) * NS + NS * (NS + 1) // 2
                    tiny_idx = [0]
                    odd_et = [None]

                    def denom_mms(src_ap, qs0):
                        for qs in range(qs0, NS):
                            nc.tensor.matmul(
                                ps_d[:, qs:qs + 1],
                                src_ap[:, qs * P:(qs + 1) * P],
                                ones_sb[:],
                                start=(tiny_idx[0] == 0),
                                stop=(tiny_idx[0] == n_tiny - 1))
                            tiny_idx[0] += 1
                            pump(1)

                    def drain_one():
                        pkc, pqlo, pet = pend.popleft()
                        nc.tensor.matmul(
                            ps_out[:, pqlo:NQ],
                            b.v[:, pkc * P:(pkc + 1) * P],
                            pet[:, pqlo:NQ],
                            start=(pkc == 0), stop=(pkc == nkc - 1))
                        # ps_d is one psum zero-region: start zeroes the
                        # whole region, so only the first write starts and
                        # only the final write stops the group.  A pump
                        # after each tiny matmul gives the next one a long
                        # matmul to hide its Ldweights behind.
                        pr = pkc - jq * NS
                        if pr < 0:
                            if odd_et[0] is None:
                                odd_et[0] = pet
                            else:
                                pair = etp.tile([P, NQ], BF16, tag="pair")
                                nc.vector.tensor_add(pair[:], odd_et[0][:],
                                                     pet[:])
                                odd_et[0] = None
                                denom_mms(pair, 0)
                        else:
                            denom_mms(pet, pr)

                    for kc in range(nkc):
                        r = kc - jq * NS
                        qlo = r * P if r >= 0 else 0
                        ps_sc = psB.tile([P, NQ], F32, tag="big")
                        nc.tensor.matmul(
                            ps_sc[:, qlo:NQ],
                            b.kT[:, kc * P:(kc + 1) * P],
                            qt[:, qlo:NQ],
                            start=True, stop=True)
                        pump(1)
                        et = etp.tile([P, NQ], BF16, tag="exp")
                        nc.scalar.activation(
                            et[:, qlo:NQ], ps_sc[:, qlo:NQ], AF.Exp,
                            scale=scale, bias=mb_sb[:, kc:kc + 1])
                        if r >= 0:
                            nc.vector.tensor_mul(
                                et[:, qlo:qlo + P], et[:, qlo:qlo + P],
                                tri_sb[:])
                        if kc == 1 and finish_prev is not None:
                            finish_prev[0]()
                        if len(pend) >= 2:
                            drain_one()
                            pump(1)
                        pend.append((kc, qlo, et))
                    while pend:
                        drain_one()
                        pump(1)
                    if finish_prev is not None:
                        # the final DVE multiply of the previous head goes
                        # here, after this head's masks, so it cannot
                        # head-of-line-block them
                        finish_prev[1]()
                    # reciprocal emitted at this head's end, ahead of the
                    # next head's rope/normalize DVE work, so the PE-side
                    # transposes in finish_a never stall on the DVE queue
                    recip = normp.tile([P, NS], BF16, tag="recip")
                    nc.vector.reciprocal(recip[:], ps_d[:])

                    state = {}

                    def finish_a():
                        # transpose each recip column to partition 0 of one
                        # [1, NQ] psum row (partition_broadcast reads p0 only)
                        psT = psB.tile([1, NQ], BF16, tag="big", name="psT")
                        for qs in range(NS):
                            # one zero-region group across the 4 transposes
                            nc.tensor.matmul(
                                psT[0:1, qs * P:(qs + 1) * P],
                                recip[:, qs:qs + 1], identb_sb[:],
                                is_transpose=True,
                                start=(qs == 0), stop=(qs == NS - 1))
                        rrow = normp.tile([1, NQ], F32, tag="rrow")
                        nc.vector.tensor_copy(rrow[:], psT[:])
                        bc = normp.tile([P, NQ], F32, tag="bc")
                        nc.gpsimd.partition_broadcast(bc[:], rrow[0:1, :])
                        state["bc"] = bc

                    def finish_b():
                        nc.vector.tensor_mul(outT_h[:], ps_out[:],
                                             state["bc"][:])
                    return finish_a, finish_b

                def gen_oproj(jq, outTs):
                    for tc4 in range(NS):
                        tok = jq * NS + tc4
                        st = stp.tile([P, H], BF16, tag="st")
                        for n in range(H // NQ):
                            ps = psB.tile([P, NQ], F32, tag="big",
                                          name="ps_op")
                            for h in range(HPC):
                                nc.tensor.matmul(
                                    ps[:],
                                    outTs[h][:, tc4 * P:(tc4 + 1) * P],
                                    wo_sb[:, h * H + n * NQ:
                                          h * H + (n + 1) * NQ],
                                    start=(h == 0), stop=(h == HPC - 1))
                                yield "ok"
                            if n % 2 == 0:
                                nc.vector.tensor_copy(
                                    st[:, n * NQ:(n + 1) * NQ], ps[:])
                            else:
                                nc.scalar.activation(
                                    st[:, n * NQ:(n + 1) * NQ], ps[:],
                                    AF.Copy)
                        nc.scalar.dma_start(
                            out_ext[tok * P:(tok + 1) * P, :], st[:])

                # ================= emission =================
                def emit_group():
                  # preamble: body 0 state + jq0 k/v/q0, dense (DMA-bound)
                  bodies[0] = alloc_body(0)
                  b0 = bodies[0]
                  xt_cur = xtp.tile([P, HC * NQ], BF16, tag="xt",
                                    name="xt_0_0")
                  load_xt_part(xt_cur, 0, 0, eng=nc.sync)
                  load_wk_wv(b0, eng=nc.scalar)
                  load_trig(b0, 0, 1, eng=nc.scalar)
                  load_xt_part(xt_cur, 0, 1, eng=nc.sync)
                  load_wq_head(b0, 0, eng=nc.scalar)
                  load_xt_part(xt_cur, 0, 2, eng=nc.sync)
                  load_wq_head(b0, 1, eng=nc.scalar)
                  load_xt_part(xt_cur, 0, 3, eng=nc.sync)

                  qts = [None] * HPC
                  fill_mid.append(gen_proj(b0.wk, 0, HD, xt_cur,
                                           rope_sink(b0, b0.kT[:, 0:NQ], 0)))
                  fill_mid.append(gen_proj(b0.wv, 0, HD, xt_cur,
                                           v_sink(b0, 0)))
                  qt0 = qtp.tile([P, NQ], BF16, tag="qt", name="qt_0_0")
                  qts[0] = qt0
                  fill_mid.append(gen_proj(b0.wq, 0, HD, xt_cur,
                                           rope_sink(b0, qt0[:], 0)))
                  flush_all()

                  finish_prev = None
                  prev_outTs = None      # (jq, outTs) awaiting o-proj
                  qts_next = [None, None]
                  next_body = None       # allocated at (u, 2) for u+1
                  for step in range(NSTEP):
                      u, jq = divmod(step, NJ)
                      bu = bodies[u % 2]
                      nxt = step + 1 < NSTEP
                      if jq == 2 and u + 1 < unroll:
                          next_body = alloc_body(u + 1)
                      xt_nxt = None
                      if nxt:
                          u1, jq1 = divmod(step + 1, NJ)
                          bt = next_body if jq1 == 0 else bu
                          xt_nxt = xtp.tile([P, HC * NQ], BF16, tag="xt",
                                            name=f"xt_{u1}_{jq1}")
                      outTs = []
                      # how much of the next q projection to emit up-front:
                      # at small jq the attention head is too short to both
                      # hide the rope latency and absorb the filler
                      prefix = 16 if jq <= 1 else 6
                      for h in range(HPC):
                          if h + 1 < HPC and qts[h + 1] is None:
                              qt = qtp.tile([P, NQ], BF16, tag="qt")
                              qts[h + 1] = qt
                              qg = gen_proj(bu.wq, (h + 1) * HC * HD, HD,
                                            xt_cur,
                                            rope_sink(bu, qt[:], jq,
                                                      on_act=jq <= 1))
                              fill_hi.append(qg)
                              pump(prefix)
                          else:
                              qg = None
                          # deferred bulk loads: current body's trig tail /
                          # wo on the SP queue (metered behind the rope rot
                          # DMAs already enqueued); next body's weights on
                          # the Act queue during jq=2/3
                          if jq == 0:
                              if h == 0:
                                  load_trig(bu, 1, NJ)
                                  if u == 0:
                                      load_wq_head(bu, 2)
                                      load_wq_head(bu, 3)
                          elif jq == 1:
                              # body u's o-projection (filler during jq>=1)
                              # reads wo; reload sits after the previous
                              # body's last wo read in the emission order
                              if h == 0:
                                  load_wo_part(0)
                                  load_wo_part(1)
                              elif h == 1:
                                  load_wo_part(2)
                                  load_wo_part(3)
                          elif jq == 2 and next_body is not None:
                              if h == 0:
                                  load_trig(next_body, 0, 1)
                              elif h == 1:
                                  load_wk_wv(next_body)
                              elif h == 2:
                                  load_wq_head(next_body, 0)
                              elif h == 3:
                                  load_wq_head(next_body, 1)
                          elif jq == 3 and next_body is not None:
                              if h == 0:
                                  load_wq_head(next_body, 2)
                                  load_wq_head(next_body, 3)
                          if nxt:
                              # queue the next step's k/v/q0 early (h=d,
                              # d+1) so their rope chains land before the
                              # next step's first score matmul needs them
                              d = 1 if jq == 0 else 0
                              if h == d:
                                  load_xt_part(xt_nxt, jq1, 0)
                                  load_xt_part(xt_nxt, jq1, 1)
                                  fill_mid.append(gen_proj(
                                      bt.wk, 0, HD, xt_nxt,
                                      rope_sink(bt,
                                                bt.kT[:, jq1 * NQ:
                                                      (jq1 + 1) * NQ],
                                                jq1, on_act=jq <= 1),
                                      gated=True))
                              elif h == d + 1:
                                  load_xt_part(xt_nxt, jq1, 2)
                                  load_xt_part(xt_nxt, jq1, 3)
                                  fill_mid.append(gen_proj(
                                      bt.wv, 0, HD, xt_nxt,
                                      v_sink(bt, jq1), gated=True))
                                  qt = qtp.tile([P, NQ], BF16, tag="qt",
                                                name=f"qt_{u1}_{jq1}")
                                  qts_next[0] = qt
                                  fill_mid.append(gen_proj(
                                      bt.wq, 0, HD, xt_nxt,
                                      rope_sink(bt, qt[:], jq1,
                                                on_act=jq <= 1),
                                      gated=True))
                          oT = outTp.tile([P, NQ], BF16, tag="oT")
                          outTs.append(oT)
                          finish_prev = attn_head(bu, h, jq, qts[h], oT,
                                                  finish_prev)
                          if h == 0 and prev_outTs is not None:
                              # previous step's o-projection becomes filler
                              # (its finish_b just ran inside attn_head)
                              fill_oq.append(gen_oproj(*prev_outTs))
                              prev_outTs = None
                          if qg is not None:
                              flush(qg)
                      # step end: next-step kv/q0 must be in the PE stream
                      # before the next step's attention matmuls
                      flush_all()
                      prev_outTs = (jq, outTs)
                      if nxt:
                          if jq == 3:
                              bodies[(u + 1) % 2] = next_body
                              next_body = None
                          xt_cur = xt_nxt
                          qts = [None] * HPC
                          qts[0] = qts_next[0]
                          qts[1] = qts_next[1]
                          qts_next = [None, None]
                  # drain the tail of the last step
                  finish_prev[0]()
                  finish_prev[1]()
                  fill_oq.append(gen_oproj(*prev_outTs))
                  flush_all()

                emit_group()
    nc.compile()
    return nc


def _host_consts():
    tri = np.triu(np.ones((P, P), dtype=BF))    # keep k_local <= q_local
    identb = np.eye(P, dtype=np.float32).astype(BF)
    onescol = np.ones((P, 1), dtype=BF)
    return tri, identb, onescol


def build_in_maps(hidden_states, cos, sin, Wq, Wk, Wv, Wo, attention_mask):
    tri, identb, onescol = _host_consts()
    cosT = np.ascontiguousarray(cos.T.astype(BF))
    sinT = sin.T.astype(np.float32)
    sinneg = np.concatenate([-sinT[:HD // 2], sinT[HD // 2:]], axis=0)
    sinnegT = np.ascontiguousarray(sinneg.astype(BF))
    in_maps = []
    for core in range(N_CORES):
        b, g = divmod(core, G)
        xT = np.ascontiguousarray(hidden_states[b].T.astype(BF))
        mb = ((attention_mask[b].astype(np.float32) - 1.0) * 1e30)
        mb = np.ascontiguousarray(mb.reshape(KC, P).T)
        wqg = Wq[:, g * HPC * HD:(g + 1) * HPC * HD].astype(BF)
        # SBUF layout [p, h*(HC*HD) + c*HD + d]
        wqh = np.ascontiguousarray(
            wqg.reshape(HC, P, HPC, HD).transpose(1, 2, 0, 3).reshape(
                P, HPC * HC * HD))
        wkh = np.ascontiguousarray(
            Wk[:, g * HD:(g + 1) * HD].astype(BF).reshape(HC, P, HD)
            .transpose(1, 0, 2).reshape(P, HC * HD))
        wvh = np.ascontiguousarray(
            Wv[:, g * HD:(g + 1) * HD].astype(BF).reshape(HC, P, HD)
            .transpose(1, 0, 2).reshape(P, HC * HD))
        in_maps.append({
            "xT": xT,
            "wq": wqh,
            "wk": wkh,
            "wv": wvh,
            "wo": np.ascontiguousarray(
                Wo[g * HPC * HD:(g + 1) * HPC * HD, :].astype(BF)),
            "cosT": cosT, "sinnegT": sinnegT,
            "tri": tri, "identb": identb,
            "mbias": mb, "onescol": onescol,
        })
    return in_maps


def kernel(hidden_states, cos, sin, Wq, Wk, Wv, Wo, attention_mask):
    if "nc" not in _CACHE:
        _CACHE["nc"] = _build_program()
    nc = _CACHE["nc"]
    in_maps = build_in_maps(np.asarray(hidden_states, np.float32),
                            np.asarray(cos, np.float32),
                            np.asarray(sin, np.float32),
                            np.asarray(Wq, np.float32),
                            np.asarray(Wk, np.float32),
                            np.asarray(Wv, np.float32),
                            np.asarray(Wo, np.float32),
                            np.asarray(attention_mask, np.float32))
    res = run_bass_kernel_spmd(nc, in_maps, list(range(N_CORES)))
    out = np.empty((B, S, H), dtype=np.float32)
    for b in range(B):
        acc = res.results[4 * b]["out_p"].astype(np.float32)
        for g in range(1, G):
            acc = acc + res.results[4 * b + g]["out_p"].astype(np.float32)
        out[b] = acc
    return out


if __name__ == "__main__":
    rng = np.random.default_rng(0)
    hs = rng.standard_normal((B, S, H), dtype=np.float32)
    inv_freq = 1.0 / (10000.0 ** (np.arange(0, HD, 2, dtype=np.float32) / HD))
    t = np.arange(S, dtype=np.float32)
    freqs = np.outer(t, inv_freq)
    emb = np.concatenate([freqs, freqs], axis=-1)
    out = kernel(hs, np.cos(emb), np.sin(emb),
                 rng.standard_normal((H, NH * HD), dtype=np.float32) * 0.02,
                 rng.standard_normal((H, NKV * HD), dtype=np.float32) * 0.02,
                 rng.standard_normal((H, NKV * HD), dtype=np.float32) * 0.02,
                 rng.standard_normal((NH * HD, H), dtype=np.float32) * 0.02,
                 np.ones((B, S), dtype=np.float32))
    print("kernel ran, out shape", out.shape, "finite:", np.isfinite(out).all())
